# revision 8
# baseline (speedup 1.0000x reference)
"""AdvancedStateBank Trainium2 kernel (8-core SPMD, token-parallel).

Each core handles 256 of the 2048 tokens; K/V banks and MLP weights are
replicated. No collectives. All precision-critical matmuls (scores, router,
predictor) run in f32 on the PE; the probs@V read matmul runs in fp32r
(4x faster, precision uncritical there). Top-64 per token via 8 rounds of
vector.max + match_replace; dynamic-k handled with a threshold mask, so the
selected set matches jax.lax.top_k + clipped-floor(sigmoid) exactly.
"""
import sys

if "/opt/trn_rl_repo" not in sys.path:
    sys.path.insert(0, "/opt/trn_rl_repo")

import numpy as np

import concourse.bacc as bacc
import concourse.mybir as mybir
from concourse.tile import TileContext
from concourse.bass_utils import run_bass_kernel_spmd
from concourse.masks import make_identity

F32 = mybir.dt.float32
F32R = mybir.dt.float32r
U8 = mybir.dt.uint8
AF = mybir.ActivationFunctionType
ALU = mybir.AluOpType
AX = mybir.AxisListType

D = 1024
B, T = 2, 1024
SLOTS = [2048, 1024, 512]
MAXK = 64
N_CORES = 8
TOK_PER_CORE = (B * T) // N_CORES  # 256
TT = TOK_PER_CORE // 128           # 2 token tiles per core
SCALE = float(1.0 / np.sqrt(np.float32(D)))
NEG_BIG = -1.0e30


def _emit_body(nc, env, rep):
    """One full inference over this core's 256 tokens."""
    cp, pp, wp, strm, sm = env["cp"], env["pp"], env["wp"], env["strm"], env["sm"]
    ps_tp, ps_s, ps_rd, ps_sm = (env["ps_tp"], env["ps_s"], env["ps_rd"],
                                 env["ps_sm"])
    ident, ones, iota_f, big = env["ident"], env["ones"], env["iota_f"], env["big"]
    rb1_row, rb2_row, pb1_row, pb2_row = (env["rb1_row"], env["rb2_row"],
                                          env["pb1_row"], env["pb2_row"])
    pW1_sb, pW2_sb, rW2_sb = env["pW1_sb"], env["pW2_sb"], env["rW2_sb"]
    qT, out_sb, route_sb, valid = (env["qT"], env["out_sb"], env["route_sb"],
                                   env["valid"])
    q, Ks, Vs, sals, rW1, out = (env["q"], env["Ks"], env["Vs"], env["sals"],
                                 env["rW1"], env["out"])

    for t in range(TT):
        nc.vector.memset(out_sb[t], 0.0)

    # ---------------- q load/transpose ----------------
    for t in range(TT):
        q_sb = wp.tile([128, D], F32, tag="qsb", bufs=2, name=f"q_{rep}_{t}")
        nc.sync.dma_start(q_sb, q[t * 128:(t + 1) * 128, :])
        for dc in range(8):
            pst = ps_tp.tile([128, 128], F32, tag="tp", name="pst")
            nc.tensor.transpose(pst, q_sb[:, dc * 128:(dc + 1) * 128], ident)
            nc.scalar.copy(qT[t][:, dc, :], pst)

    # ---------------- router: h1 = gelu(q @ rW1 + rb1) ----------------
    # h1 in [T, Dout] orientation (rW1 used as stored, streamed), then
    # PE-transposed to h1T for the second matmul.
    ph1 = {}
    for t in range(TT):
        for c2 in range(2):
            ph1[(t, c2)] = ps_rd.tile([128, 512], F32, tag=f"rd{t}{c2}",
                                      name=f"ph1_{rep}_{t}{c2}", bufs=1)
    for ci in range(8):
        rwc = strm.tile([128, D], F32, tag="stream", bufs=4,
                        name=f"rwc_{rep}_{ci}")
        nc.sync.dma_start(rwc, rW1[ci * 128:(ci + 1) * 128, :])
        for t in range(TT):
            for c2 in range(2):
                nc.tensor.matmul(ph1[(t, c2)], qT[t][:, ci, :],
                                 rwc[:, c2 * 512:(c2 + 1) * 512],
                                 start=(ci == 0), stop=False)
    for t in range(TT):
        for c2 in range(2):
            nc.tensor.matmul(ph1[(t, c2)], ones,
                             rb1_row[0:1, c2 * 512:(c2 + 1) * 512],
                             start=False, stop=True)
    for t in range(TT):
        h1_sb = wp.tile([128, D], F32, tag=f"scores{t}", name=f"h1_{rep}_{t}")
        for c2 in range(2):
            nc.scalar.activation(h1_sb[:, c2 * 512:(c2 + 1) * 512],
                                 ph1[(t, c2)], AF.Gelu)
        h1T = wp.tile([128, 8, 128], F32, tag=f"scratch{t}",
                      name=f"h1T_{rep}_{t}")
        for dc in range(8):
            pst = ps_tp.tile([128, 128], F32, tag="tp", name="pst")
            nc.tensor.transpose(pst, h1_sb[:, dc * 128:(dc + 1) * 128], ident)
            nc.scalar.copy(h1T[:, dc, :], pst)
        # route logits [T,3] + softmax over 3
        pr = ps_sm.tile([128, 3], F32, tag="ps_small", name="pr")
        for co in range(8):
            nc.tensor.matmul(pr, h1T[:, co, :], rW2_sb[:, co, :],
                             start=(co == 0), stop=False)
        nc.tensor.matmul(pr, ones, rb2_row, start=False, stop=True)
        m3 = sm.tile([128, 1], F32, tag="m3", name="m3")
        nc.vector.tensor_reduce(m3, pr, axis=AX.X, op=ALU.max)
        negm3 = sm.tile([128, 1], F32, tag="negm3", name="negm3")
        nc.vector.tensor_scalar_mul(negm3, m3, -1.0)
        e3 = sm.tile([128, 3], F32, tag="e3", name="e3")
        nc.scalar.activation(e3, pr, AF.Exp, bias=negm3[:, 0:1])
        s3 = sm.tile([128, 1], F32, tag="s3", name="s3")
        nc.vector.tensor_reduce(s3, e3, axis=AX.X, op=ALU.add)
        inv3 = sm.tile([128, 1], F32, tag="inv3", name="inv3")
        nc.vector.reciprocal(inv3, s3)
        nc.vector.tensor_scalar(route_sb[t], e3, inv3[:, 0:1], None,
                                op0=ALU.mult)

        # predictor: p1T = gelu(pW1^T @ q^T + pb1 x ones)  [64, T]
        pps = ps_sm.tile([64, 128], F32, tag="ps_small", name="pps")
        for ci in range(8):
            nc.tensor.matmul(pps, pW1_sb[:, ci, :], qT[t][:, ci, :],
                             start=(ci == 0), stop=False)
        nc.tensor.matmul(pps, pb1_row, ones, start=False, stop=True)
        p1T = sm.tile([64, 128], F32, tag="p1T", name="p1T")
        nc.scalar.activation(p1T, pps, AF.Gelu)
        pl = ps_sm.tile([128, 1], F32, tag="ps_small", name="pl")
        nc.tensor.matmul(pl, p1T, pW2_sb, start=True, stop=False)
        nc.tensor.matmul(pl, ones, pb2_row, start=False, stop=True)
        sg = sm.tile([128, 1], F32, tag="sg", name="sg")
        nc.scalar.activation(sg, pl, AF.Sigmoid)
        kx64 = sm.tile([128, 1], F32, tag="kx64", name="kx64")
        nc.vector.tensor_scalar_mul(kx64, sg, float(MAXK))
        # valid[j] = (iota_f[j] <= kx64); col0 = -1e30 => always true
        nc.vector.tensor_scalar(valid[t], iota_f, kx64[:, 0:1], None,
                                op0=ALU.is_le)

    # ---------------- levels ----------------
    for li, S in enumerate(SLOTS):
        nsc = S // 128   # 128-wide slot chunks
        nch = S // 512   # 512-wide score chunks

        sal_row = pp.tile([1, SLOTS[0]], F32, tag="sal", name=f"sal_{rep}_{li}")
        nc.sync.dma_start(sal_row[0:1, :S], sals[li].unsqueeze(0))

        scores_t = [wp.tile([128, S], F32, tag=f"scores{t}",
                            name=f"sco_{rep}_{li}_{t}") for t in range(TT)]

        # scores, with K transposed on the fly in 512-slot blocks
        for ch in range(nch):
            ktb = pp.tile([128, 8, 512], F32, tag="ktb", name="ktb", bufs=2)
            for s4 in range(4):
                ksb = strm.tile([128, D], F32, tag="stream", bufs=4,
                                name="ksb")
                nc.sync.dma_start(
                    ksb,
                    Ks[li][ch * 512 + s4 * 128:ch * 512 + (s4 + 1) * 128, :])
                for dc in range(8):
                    pst = ps_tp.tile([128, 128], F32, tag="tp", name="pst")
                    nc.tensor.transpose(pst, ksb[:, dc * 128:(dc + 1) * 128],
                                        ident)
                    nc.scalar.mul(ktb[:, dc, s4 * 128:(s4 + 1) * 128], pst,
                                  SCALE)
            for t in range(TT):
                pss = ps_s.tile([128, 512], F32, tag="ps_s", bufs=1,
                                name="pss")
                for ci in range(8):
                    nc.tensor.matmul(pss, qT[t][:, ci, :], ktb[:, ci, :],
                                     start=(ci == 0), stop=False)
                nc.tensor.matmul(pss, ones,
                                 sal_row[0:1, ch * 512:(ch + 1) * 512],
                                 start=False, stop=True)
                nc.scalar.copy(scores_t[t][:, ch * 512:(ch + 1) * 512], pss)

        probsT = []
        ws = []
        for t in range(TT):
            scores = scores_t[t]
            # top-64 (descending) via 8 rounds of max + match_replace
            top64 = sm.tile([128, 64], F32, tag=f"top64_{t}",
                            name=f"top64_{t}")
            scratch = wp.tile([128, S], F32, tag=f"scratch{t}",
                              name=f"scr_{rep}_{li}_{t}")
            nc.vector.max(out=top64[:, 0:8], in_=scores)
            nc.vector.match_replace(out=scratch, in_to_replace=top64[:, 0:8],
                                    in_values=scores, imm_value=NEG_BIG)
            for r in range(1, 8):
                nc.vector.max(out=top64[:, r * 8:(r + 1) * 8], in_=scratch)
                if r < 7:
                    nc.vector.match_replace(
                        out=scratch, in_to_replace=top64[:, r * 8:(r + 1) * 8],
                        in_values=scratch, imm_value=NEG_BIG)

            # t_sel = value at rank dyn_k (smallest selected score)
            selv = sm.tile([128, 64], F32, tag="selv", name="selv")
            nc.vector.select(selv, valid[t], top64, big)
            t_sel = sm.tile([128, 1], F32, tag="t_sel", name="t_sel")
            nc.vector.tensor_reduce(t_sel, selv, axis=AX.X, op=ALU.min)
            negm = sm.tile([128, 1], F32, tag="negm", name="negm")
            nc.vector.tensor_scalar_mul(negm, top64[:, 0:1], -1.0)

            e_sb = wp.tile([128, S], F32, tag="esb", bufs=2,
                           name=f"e_{rep}_{li}_{t}")
            nc.scalar.activation(e_sb, scores, AF.Exp, bias=negm[:, 0:1])

            # probs = (scores >= t_sel) * e ; denom = sum(probs)
            denom = sm.tile([128, 1], F32, tag="denom", name="denom")
            nc.vector.scalar_tensor_tensor(
                out=scratch, in0=scores, scalar=t_sel[:, 0:1], in1=e_sb,
                op0=ALU.is_ge, op1=ALU.mult, accum_out=denom[:, 0:1])
            rden = sm.tile([128, 1], F32, tag="rden", name="rden")
            nc.vector.reciprocal(rden, denom)
            w = sm.tile([128, 1], F32, tag=f"w{t}", name=f"w{t}")
            nc.vector.tensor_tensor(out=w, in0=rden,
                                    in1=route_sb[t][:, li:li + 1],
                                    op=ALU.mult)
            ws.append(w)

            # probsT (rounded to f32r on psum->sbuf copy)
            pT = pp.tile([128, SLOTS[0] // 128, 128], F32R, tag=f"pt{t}",
                         name=f"pT{t}")
            for sc in range(nsc):
                pst = ps_tp.tile([128, 128], F32, tag="tp", name="pst")
                nc.tensor.transpose(pst, scratch[:, sc * 128:(sc + 1) * 128],
                                    ident)
                nc.vector.tensor_copy(pT[:, sc, :], pst)
            probsT.append(pT)

        # read matmul: out[T,D] += probs @ V, fp32r, V streamed
        prd = {}
        for t in range(TT):
            for dc2 in range(2):
                prd[(t, dc2)] = ps_rd.tile([128, 512], F32, tag=f"rd{t}{dc2}",
                                           name=f"prd{t}{dc2}")
        for sc in range(nsc):
            vsb = strm.tile([128, D], F32, tag="stream", bufs=4, name="vsb")
            nc.sync.dma_start(vsb, Vs[li][sc * 128:(sc + 1) * 128, :])
            vr = strm.tile([128, D], F32R, tag="vr", bufs=3, name="vr")
            nc.gpsimd.tensor_copy(vr, vsb)
            for t in range(TT):
                for dc2 in range(2):
                    nc.tensor.matmul(prd[(t, dc2)], probsT[t][:, sc, :],
                                     vr[:, dc2 * 512:(dc2 + 1) * 512],
                                     start=(sc == 0), stop=(sc == nsc - 1))
        for t in range(TT):
            for dc2 in range(2):
                seg = slice(dc2 * 512, (dc2 + 1) * 512)
                nc.vector.scalar_tensor_tensor(
                    out=out_sb[t][:, seg], in0=prd[(t, dc2)],
                    scalar=ws[t][:, 0:1], in1=out_sb[t][:, seg],
                    op0=ALU.mult, op1=ALU.add)

    for t in range(TT):
        nc.sync.dma_start(out[t * 128:(t + 1) * 128, :], out_sb[t])


def build_nc(repeat=1):
    nc = bacc.Bacc(trn_type="TRN2", debug=False)

    env = {}
    env["q"] = nc.dram_tensor("q", [TOK_PER_CORE, D], F32,
                              kind="ExternalInput").ap()
    env["Ks"], env["Vs"], env["sals"] = [], [], []
    for i, S in enumerate(SLOTS):
        env["Ks"].append(
            nc.dram_tensor(f"K{i}", [S, D], F32, kind="ExternalInput").ap())
        env["Vs"].append(
            nc.dram_tensor(f"V{i}", [S, D], F32, kind="ExternalInput").ap())
        env["sals"].append(
            nc.dram_tensor(f"sal{i}", [S], F32, kind="ExternalInput").ap())
    env["rW1"] = nc.dram_tensor("rW1", [D, D], F32, kind="ExternalInput").ap()
    rb1 = nc.dram_tensor("rb1", [D], F32, kind="ExternalInput").ap()
    rW2 = nc.dram_tensor("rW2", [D, 3], F32, kind="ExternalInput").ap()
    rb2 = nc.dram_tensor("rb2", [3], F32, kind="ExternalInput").ap()
    pW1 = nc.dram_tensor("pW1", [D, 64], F32, kind="ExternalInput").ap()
    pb1 = nc.dram_tensor("pb1", [64], F32, kind="ExternalInput").ap()
    pW2 = nc.dram_tensor("pW2", [64, 1], F32, kind="ExternalInput").ap()
    pb2 = nc.dram_tensor("pb2", [1], F32, kind="ExternalInput").ap()
    env["out"] = nc.dram_tensor("out", [TOK_PER_CORE, D], F32,
                                kind="ExternalOutput").ap()

    with TileContext(nc) as tc:
        with (
            tc.tile_pool(name="const", bufs=1) as cp,
            tc.tile_pool(name="persist", bufs=1) as pp,
            tc.tile_pool(name="work", bufs=1) as wp,
            tc.tile_pool(name="stream", bufs=1) as strm,
            tc.tile_pool(name="small", bufs=2) as sm,
            tc.tile_pool(name="ps_tp", bufs=2, space="PSUM") as ps_tp,
            tc.tile_pool(name="ps_s", bufs=1, space="PSUM") as ps_s,
            tc.tile_pool(name="ps_rd", bufs=1, space="PSUM") as ps_rd,
            tc.tile_pool(name="ps_sm", bufs=1, space="PSUM") as ps_sm,
        ):
            env.update(cp=cp, pp=pp, wp=wp, strm=strm, sm=sm, ps_tp=ps_tp,
                       ps_s=ps_s, ps_rd=ps_rd, ps_sm=ps_sm)
            # ---------------- constants ----------------
            ident = cp.tile([128, 128], F32)
            make_identity(nc, ident)
            ones = cp.tile([1, 128], F32)
            nc.vector.memset(ones, 1.0)
            iota_i = cp.tile([128, 64], mybir.dt.int32)
            nc.gpsimd.iota(iota_i, pattern=[[1, 64]], base=1,
                           channel_multiplier=0)
            iota_f = cp.tile([128, 64], F32)
            nc.vector.tensor_copy(iota_f, iota_i)
            nc.vector.memset(iota_f[:, 0:1], NEG_BIG)
            big = cp.tile([128, 64], F32)
            nc.vector.memset(big, 1.0e30)

            rb1_row = cp.tile([1, D], F32)
            nc.sync.dma_start(rb1_row, rb1.unsqueeze(0))
            rb2_row = cp.tile([1, 3], F32)
            nc.sync.dma_start(rb2_row, rb2.unsqueeze(0))
            pb1_row = cp.tile([1, 64], F32)
            nc.sync.dma_start(pb1_row, pb1.unsqueeze(0))
            pb2_row = cp.tile([1, 1], F32)
            nc.sync.dma_start(pb2_row, pb2.unsqueeze(0))

            pW1_sb = cp.tile([128, 8, 64], F32)
            nc.sync.dma_start(pW1_sb, pW1.rearrange("(c p) o -> p c o", p=128))
            pW2_sb = cp.tile([64, 1], F32)
            nc.sync.dma_start(pW2_sb, pW2)
            rW2_sb = cp.tile([128, 8, 3], F32)
            nc.sync.dma_start(rW2_sb, rW2.rearrange("(c p) o -> p c o", p=128))

            env.update(ident=ident, ones=ones, iota_f=iota_f, big=big,
                       rb1_row=rb1_row, rb2_row=rb2_row, pb1_row=pb1_row,
                       pb2_row=pb2_row, pW1_sb=pW1_sb, pW2_sb=pW2_sb,
                       rW2_sb=rW2_sb)

            # persistent per-token-tile state
            env["qT"] = [pp.tile([128, 8, 128], F32, tag=f"qT{t}",
                                 name=f"qT{t}") for t in range(TT)]
            env["out_sb"] = [pp.tile([128, D], F32, tag=f"out{t}",
                                     name=f"out_sb{t}") for t in range(TT)]
            env["route_sb"] = [pp.tile([128, 3], F32, tag=f"route{t}",
                                       name=f"route{t}") for t in range(TT)]
            env["valid"] = [pp.tile([128, 64], U8, tag=f"valid{t}",
                                    name=f"valid{t}") for t in range(TT)]

            for rep in range(repeat):
                _emit_body(nc, env, rep)

    nc.compile()
    return nc


_NC_CACHE = None


def _get_nc():
    global _NC_CACHE
    if _NC_CACHE is None:
        _NC_CACHE = build_nc()
    return _NC_CACHE


def make_in_maps(inputs):
    q_full = np.ascontiguousarray(
        np.asarray(inputs["q"], dtype=np.float32).reshape(B * T, D))
    shared = {}
    for name in ["K0", "V0", "sal0", "K1", "V1", "sal1", "K2", "V2", "sal2",
                 "rW1", "rb1", "rW2", "rb2", "pW1", "pb1", "pW2", "pb2"]:
        shared[name] = np.ascontiguousarray(
            np.asarray(inputs[name], dtype=np.float32))
    in_maps = []
    for c in range(N_CORES):
        m = dict(shared)
        m["q"] = np.ascontiguousarray(
            q_full[c * TOK_PER_CORE:(c + 1) * TOK_PER_CORE])
        in_maps.append(m)
    return in_maps


def kernel(**inputs):
    nc = _get_nc()
    in_maps = make_in_maps(inputs)
    res = run_bass_kernel_spmd(nc, in_maps, core_ids=list(range(N_CORES)))
    out = np.concatenate([res.results[c]["out"] for c in range(N_CORES)],
                         axis=0)
    return out.reshape(B, T, D)
